# revision 8
# baseline (speedup 1.0000x reference)
"""Category-specific MLP (MoE-style routing) for Trainium2, 8 NeuronCores.

Reference computation (per token n):
    h   = relu(x[n] @ W1[cat[n]] + b1[cat[n]])      x:[N,128]  W1:[100,128,128]
    out = h @ W2[cat[n]] + b2[cat[n]]               W2:[100,128,64]

Strategy (expert-parallel, MoE-style):
  * Host: sort tokens by category; one work item per category (split at 512).
    Items sorted by size desc, item r -> (core r%8, slot r//8). SPMD: all
    cores run the same program; slot s has fixed capacity caps[s] (max item
    size in that slot across cores) so shapes match everywhere (~5% pad).
  * Feature-on-partitions layout. Slots are packed in processing order into
    chunks of <= CHUNK_COLS token-columns; each chunk is one contiguous
    [128, W_c] fp16 block in DRAM (per chunk: w1 ns*128 | w2 ns*64 | xT) and
    one HWDGE DMA, alternating between the SP and ACT rings so both rings
    stream concurrently.
    Per slot (fp16 matmuls, fp32 PSUM accumulate):
        psum1[:, lo:lo+B] = W1_s^T @ xT_s        (PE)
        psum2[:, lo:lo+B] = W2_s^T @ h_s         (PE)
    Per chunk: relu evacuation psum1 -> h (fp16 SBUF) alternates between the
    DVE and ACT engines so evacuation is not serialized on one engine; the
    fp32 result is stored straight out of PSUM (no cast/copy op).
    Chunks are software-pipelined (layer 2 of chunk c emitted after layer 1
    of chunk c+1) so the PE stream never stalls on an evacuation.
  * Host: scatter outT columns back to original token order.

fp16 numerics: fp16 inputs (10-bit mantissa), fp32 PSUM accumulation.
Measured vs fp32 reference: absmax-relative error ~6e-4.
"""

from contextlib import ExitStack

import numpy as np

import concourse.bass as bass
import concourse.mybir as mybir
import concourse.tile as tile
from concourse import bacc
from concourse.bass_utils import run_bass_kernel_spmd

N, C, D, H, O = 8192, 100, 128, 128, 64
NCORES = 8
MAX_ITEM = 512      # PSUM bank / moving-operand limit (fp32 columns)
CHUNK_COLS = 512    # token-column budget per chunk (<= one PSUM bank)

F16 = mybir.dt.float16
F32 = mybir.dt.float32


def _plan(cat_ids: np.ndarray, zero_bias: bool):
    """Host-side routing plan: work items -> (core, slot), slots -> chunks."""
    cat_ids = np.asarray(cat_ids).astype(np.int64)
    counts = np.bincount(cat_ids, minlength=C)
    NC = len(counts)                                    # robust to ids >= C
    order = np.argsort(cat_ids, kind="stable")          # token ids sorted by cat
    starts = np.zeros(NC, dtype=np.int64)
    starts[1:] = np.cumsum(counts)[:-1]

    items = []                                          # (cat, start_in_cat, len)
    for c in range(NC):
        cnt = int(counts[c])
        o = 0
        while o < cnt:
            ln = min(MAX_ITEM, cnt - o)
            items.append((c, o, ln))
            o += ln
    items.sort(key=lambda it: -it[2])

    S = (len(items) + NCORES - 1) // NCORES
    grid = [[None] * NCORES for _ in range(S)]          # grid[s][k] = item|None
    for r, it in enumerate(items):
        grid[r // NCORES][r % NCORES] = it
    caps = tuple(max(1, max((it[2] for it in row if it is not None), default=1))
                 for row in grid)
    offs = np.zeros(S + 1, dtype=np.int64)
    offs[1:] = np.cumsum(caps)
    T = int(offs[-1])

    # Pack slots in processing order into chunks of <= CHUNK_COLS columns.
    chunks = []                                         # (s0, s1)
    s0 = 0
    while s0 < S:
        s1 = s0 + 1
        while s1 < S and int(offs[s1 + 1] - offs[s0]) <= CHUNK_COLS:
            s1 += 1
        chunks.append((s0, s1))
        s0 = s1
    G = len(chunks)

    def gwidth(gi):
        s0, s1 = chunks[gi]
        return (s1 - s0) * (H + O) + int(offs[s1] - offs[s0])

    # Each chunk is one contiguous [128, W_g] row-major fp16 block; chunk gi
    # rides ring gi % 2 (0 = SP/sync, 1 = ACT/scalar) so consecutive chunks
    # stream on different rings concurrently while per-ring FIFO order keeps
    # completions in processing order.
    bpos = {}
    pos = 0
    for gi in range(G):
        bpos[gi] = pos
        pos += 128 * gwidth(gi)
    Z = pos

    return {
        "order": order, "starts": starts, "grid": grid,
        "S": S, "caps": caps, "offs": offs, "T": T,
        "chunks": chunks, "Z": Z, "zero_bias": zero_bias, "bpos": bpos,
        "gwidth": {gi: gwidth(gi) for gi in range(G)},
    }


_NC_CACHE: dict = {}


def _build_nc(plan):
    S, caps, T, Z = plan["S"], plan["caps"], plan["T"], plan["Z"]
    zero_bias = plan["zero_bias"]
    key = (S, caps, zero_bias)
    if key in _NC_CACHE:
        return _NC_CACHE[key]

    offs, chunks = plan["offs"], plan["chunks"]
    bpos, gw = plan["bpos"], plan["gwidth"]
    G = len(chunks)

    nc = bacc.Bacc("TRN2", target_bir_lowering=False, debug=False,
                   enable_partition_id=False)
    blob_d = nc.dram_tensor("blob", [Z], F16, kind="ExternalInput").ap()
    if not zero_bias:
        bias_d = nc.dram_tensor("bias", [128, 2 * S], F32,
                                kind="ExternalInput").ap()
    out_d = nc.dram_tensor("outT", [O * T], F16, kind="ExternalOutput").ap()

    rings = [None, None]  # filled inside tc: [sync, scalar]

    with tile.TileContext(nc) as tc, ExitStack() as ctx:
        rings[0], rings[1] = nc.sync, nc.scalar

        loads = ctx.enter_context(tc.tile_pool(name="loads", bufs=1))
        hbuf = ctx.enter_context(tc.tile_pool(name="hbuf", bufs=2))
        obuf = ctx.enter_context(tc.tile_pool(name="obuf", bufs=2))
        ps1p = ctx.enter_context(tc.tile_pool(name="ps1p", bufs=2, space="PSUM"))
        ps2p = ctx.enter_context(tc.tile_pool(name="ps2p", bufs=2, space="PSUM"))

        gtiles = {}
        for gi in range(G):
            g_sb = loads.tile([128, gw[gi]], F16, tag=f"blk_{gi}",
                              name=f"blk_{gi}")
            rings[gi % 2].dma_start(
                out=g_sb,
                in_=blob_d[bpos[gi]:bpos[gi] + 128 * gw[gi]]
                .rearrange("(p w) -> p w", p=128))
            gtiles[gi] = g_sb

        if not zero_bias:
            consts = ctx.enter_context(tc.tile_pool(name="consts", bufs=1))
            bias = consts.tile([128, 2 * S], F32)
            nc.sync.dma_start(out=bias, in_=bias_d)

        state = {}      # per live chunk: h tile for the layer-2 phase

        def phase1(gi):
            s0, s1 = chunks[gi]
            co0 = int(offs[s0])
            cols = int(offs[s1]) - co0
            blk = gtiles[gi]
            ns = s1 - s0
            xv_base = ns * (H + O)
            ps1 = ps1p.tile([H, cols], F32, tag="ps1", name=f"ps1_{gi}")
            for s in range(s0, s1):
                i, B = s - s0, int(caps[s])
                lo = int(offs[s]) - co0
                nc.tensor.matmul(ps1[:, lo:lo + B],
                                 lhsT=blk[:, i * H:(i + 1) * H],
                                 rhs=blk[:, xv_base + lo:xv_base + lo + B],
                                 start=True, stop=True)
            h_g = hbuf.tile([H, cols], F16, tag="h", name=f"h_{gi}")
            if gi % 2 == 1:
                # ACT engine: out = relu(in + b1) with per-partition bias AP
                if zero_bias:
                    nc.scalar.activation(h_g, ps1,
                                         mybir.ActivationFunctionType.Relu)
                else:
                    for s in range(s0, s1):
                        B = int(caps[s])
                        lo = int(offs[s]) - co0
                        nc.scalar.activation(
                            h_g[:, lo:lo + B], ps1[:, lo:lo + B],
                            mybir.ActivationFunctionType.Relu,
                            bias=bias[:, s:s + 1])
            else:
                if zero_bias:
                    nc.vector.tensor_scalar_max(h_g, ps1, 0.0)
                else:
                    for s in range(s0, s1):
                        B = int(caps[s])
                        lo = int(offs[s]) - co0
                        nc.vector.tensor_scalar(
                            h_g[:, lo:lo + B], ps1[:, lo:lo + B],
                            bias[:, s:s + 1],
                            0.0, mybir.AluOpType.add, mybir.AluOpType.max)
            state[gi] = h_g

        def phase2(gi):
            s0, s1 = chunks[gi]
            co0, co1 = int(offs[s0]), int(offs[s1])
            cols = co1 - co0
            h_g = state.pop(gi)
            blk = gtiles[gi]
            ns = s1 - s0
            w2_base = ns * H
            ps2 = ps2p.tile([O, cols], F32, tag="ps2", name=f"ps2_{gi}")
            for s in range(s0, s1):
                i, B = s - s0, int(caps[s])
                lo = int(offs[s]) - co0
                nc.tensor.matmul(ps2[:, lo:lo + B],
                                 lhsT=blk[:, w2_base + i * O:w2_base + (i + 1) * O],
                                 rhs=h_g[:, lo:lo + B], start=True, stop=True)
            # evacuate psum2 on the engine NOT doing this pipeline stage's
            # relu (relu of chunk gi+1 runs on engine (gi+1)%2)
            o_g = obuf.tile([O, cols], F16, tag="o", name=f"o_{gi}")
            if zero_bias:
                if gi % 2 == 0:
                    nc.scalar.copy(o_g, ps2)
                else:
                    nc.vector.tensor_copy(o_g, ps2)
            else:
                for s in range(s0, s1):
                    B = int(caps[s])
                    lo = int(offs[s]) - co0
                    if gi % 2 == 0:
                        nc.scalar.add(o_g[:, lo:lo + B], ps2[:, lo:lo + B],
                                      bias[0:O, S + s:S + s + 1])
                    else:
                        nc.vector.tensor_scalar_add(o_g[:, lo:lo + B],
                                                    ps2[:, lo:lo + B],
                                                    bias[0:O, S + s:S + s + 1])
            dst = out_d[O * co0:O * co1].rearrange("(p w) -> p w", p=O)
            # ring opposite to the chunk's load ring so the store doesn't
            # queue behind a pending load on the same HWDGE FIFO
            rings[(gi + 1) % 2].dma_start(out=dst, in_=o_g)

        # software pipeline: layer-2 of chunk g rides behind layer-1 of g+1
        phase1(0)
        for i in range(1, G):
            phase1(i)
            phase2(i - 1)
        phase2(G - 1)

    nc.compile()
    _NC_CACHE[key] = nc
    return nc


def _shard_inputs(x, W1, b1, W2, b2, plan):
    S, offs, Z = plan["S"], plan["offs"], plan["Z"]
    order, starts, grid = plan["order"], plan["starts"], plan["grid"]
    chunks, bpos = plan["chunks"], plan["bpos"]

    x16 = x.astype(np.float16)
    W116 = W1.astype(np.float16)
    W216 = W2.astype(np.float16)

    in_maps = []
    for k in range(NCORES):
        blob = np.zeros(Z, dtype=np.float16)
        if not plan["zero_bias"]:
            biasc = np.zeros((128, 2 * S), dtype=np.float32)
        for gi, (s0, s1) in enumerate(chunks):
            ns = s1 - s0
            co0 = int(offs[s0])
            cols = int(offs[s1]) - co0
            W_g = ns * (H + O) + cols
            gb = blob[bpos[gi]:bpos[gi] + 128 * W_g].reshape(128, W_g)
            for s in range(s0, s1):
                it = grid[s][k]
                if it is None:
                    continue
                i = s - s0
                c, o, ln = it
                toks = order[starts[c] + o: starts[c] + o + ln]
                gb[:, i * H:(i + 1) * H] = W116[c]
                gb[:, ns * H + i * O:ns * H + (i + 1) * O] = W216[c]
                xoff = ns * (H + O) + (int(offs[s]) - co0)
                gb[:, xoff:xoff + ln] = x16[toks].T
                if not plan["zero_bias"]:
                    biasc[:, s] = b1[c]
                    biasc[0:O, S + s] = b2[c]
        m = {"blob": blob}
        if not plan["zero_bias"]:
            m["bias"] = biasc
        in_maps.append(m)
    return in_maps


def _unshard(results, plan):
    offs, T = plan["offs"], plan["T"]
    order, starts, grid = plan["order"], plan["starts"], plan["grid"]
    chunks = plan["chunks"]
    out = np.empty((N, O), dtype=np.float32)
    for k in range(NCORES):
        flat = results[k]["outT"].astype(np.float32)
        for (s0, s1) in chunks:
            co0, co1 = int(offs[s0]), int(offs[s1])
            blk = flat[O * co0:O * co1].reshape(O, co1 - co0)
            for s in range(s0, s1):
                it = grid[s][k]
                if it is None:
                    continue
                c, o, ln = it
                toks = order[starts[c] + o: starts[c] + o + ln]
                lo = int(offs[s]) - co0
                out[toks] = blk[:, lo:lo + ln].T
    return out


def _execute(x, cat_ids, W1, b1, W2, b2, trace=False):
    x = np.asarray(x, dtype=np.float32)
    W1 = np.asarray(W1, dtype=np.float32)
    b1 = np.asarray(b1, dtype=np.float32)
    W2 = np.asarray(W2, dtype=np.float32)
    b2 = np.asarray(b2, dtype=np.float32)

    zero_bias = not (b1.any() or b2.any())
    plan = _plan(cat_ids, zero_bias)
    nc = _build_nc(plan)
    in_maps = _shard_inputs(x, W1, b1, W2, b2, plan)
    res = run_bass_kernel_spmd(nc, in_maps, list(range(NCORES)), trace=trace)
    out = _unshard(res.results, plan)
    return out, res


def kernel(x, cat_ids, W1, b1, W2, b2):
    out, _ = _execute(x, cat_ids, W1, b1, W2, b2, trace=False)
    return out
